# revision 1
# baseline (speedup 1.0000x reference)
"""ConvCapsuleLayer Trainium2 kernel.

Per-core (data-parallel over batch B=8 across 8 NeuronCores):
  * 5x5 conv (32->256 ch, 64x64, pad 2) as 8 accumulating K=128 matmuls per
    128-pixel block per image, with the (cin x 4-tap) contraction packed onto
    the 128 PE partitions.  Taps are baked in by staging 4 shifted copies of
    the padded input on partitions (host prep).  A 9th "image" holding
    sum_ic(x) yields sum_ic(votes) by conv linearity, which kills the
    routing-iteration-1 weighted-sum passes (route is exactly 1/8).
  * 3 dynamic-routing iterations fused per 128-pixel block, fp32 throughout
    (bf16 anywhere in the votes path blows up the logits; measured 6.6% L2).
    Layout: partitions = pixels, free = (ic, nc, na); reduces via strided
    innermost-axis tensor_reduce; broadcasts via step-0 APs.
  * Output transposed to [256, pix] via PE transpose, DMA'd to [NC,NA,H,W].
"""

import sys

sys.path.insert(0, "/opt/trn_rl_repo")

import numpy as np

import concourse.bass as bass
import concourse.tile as tile
from concourse import bacc, mybir
from concourse.bass_utils import run_bass_kernel_spmd

F32 = mybir.dt.float32
AX = mybir.AxisListType
OP = mybir.AluOpType
AF = mybir.ActivationFunctionType

B = 8
IC = 8
CIN = 32
NC_ = 8
NA = 32
COUT = NC_ * NA  # 256
H = 64
WD = 64
K = 5
PAD = 2
PW = H + 2 * PAD  # 68
NIMG = IC + 1  # 8 images + sum image
NG = 8  # matmul tap-groups per image
NBLK = 32  # pixel blocks (2 rows of 64 each)
BLKP = 128  # pixels per block

# 4-tap partition shape (dr,dc) and the 8 group translates (a,b).
# Covers all 25 taps of the 5x5 kernel: tap (t,j) = SHAPE[t] + TRANS[j].
SHAPE_T = [(0, 0), (0, 1), (2, 1), (3, 1)]
TRANS = [(0, 0), (0, 2), (1, 1), (1, 3), (1, -1), (0, 3), (2, 0), (2, 2)]

# per-image staged stream length: the full padded image (matmul patch
# slices top out at 62*68 + 138 + 136 = 4490 < 4624).
SLK = PW * PW  # 4624


def _tap_assignment():
    """(t, j) -> (kh, kw) assignment; each of the 25 taps exactly once."""
    assign = {}
    for j, (a, b_) in enumerate(TRANS):
        for t, (dr, dc) in enumerate(SHAPE_T):
            kh, kw = dr + a, dc + b_
            if 0 <= kh < K and 0 <= kw < K and (kh, kw) not in assign:
                assign[(kh, kw)] = (t, j)
    assert len(assign) == K * K, f"tap cover incomplete: {len(assign)}"
    return assign


def _host_prep(input_tensor, W, b):
    x = np.asarray(input_tensor, dtype=np.float32)
    W = np.asarray(W, dtype=np.float32)
    b = np.asarray(b, dtype=np.float32)

    # Padded flat images incl. the sum image, then 4 shifted copies on the
    # partition axis: xr[b, img, t*32+c, i] = xpad[b, img, c, i + s_t].
    xpad = np.zeros((B, NIMG, CIN, PW, PW), np.float32)
    xpad[:, :IC, :, PAD : PAD + H, PAD : PAD + WD] = x
    xpad[:, IC] = xpad[:, :IC].sum(axis=1)
    xflat = np.zeros((B, NIMG, CIN, SLK), np.float32)
    xflat[:, :, :, : PW * PW] = xpad.reshape(B, NIMG, CIN, PW * PW)

    xr = np.zeros((B, NIMG, 128, SLK), np.float32)
    for t, (dr, dc) in enumerate(SHAPE_T):
        s = dr * PW + dc
        xr[:, :, t * CIN : (t + 1) * CIN, : SLK - s] = xflat[:, :, :, s:]

    # Packed weights: Wp[j, t*32+c, co] = W[co, c, kh, kw] for the assigned
    # (t, j) -> (kh, kw); zero elsewhere.
    assign = _tap_assignment()
    Wp = np.zeros((NG, 128, COUT), np.float32)
    for (kh, kw), (t, j) in assign.items():
        Wp[j, t * CIN : (t + 1) * CIN, :] = W[:, :, kh, kw].T

    bias = np.ascontiguousarray(
        np.broadcast_to(b, (1, 1, NC_, NA)).reshape(COUT), dtype=np.float32
    )
    ident = np.eye(128, dtype=np.float32)
    return xr, Wp, bias, ident


_PROGRAM = None


def _build_program():
    nc = bacc.Bacc("TRN2", target_bir_lowering=False, debug=False, num_devices=8)
    xr_d = nc.dram_tensor("xr", [NIMG, 128, SLK], F32, kind="ExternalInput")
    wp_d = nc.dram_tensor("wp", [NG, 128, COUT], F32, kind="ExternalInput")
    bias_d = nc.dram_tensor("bias", [COUT], F32, kind="ExternalInput")
    id_d = nc.dram_tensor("ident", [128, 128], F32, kind="ExternalInput")
    y_d = nc.dram_tensor("y", [COUT, H * WD], F32, kind="ExternalOutput")

    with tile.TileContext(nc) as tc:
        _emit(nc, tc, xr_d, wp_d, bias_d, id_d, y_d)
    nc.compile()
    return nc


def _emit(nc, tc, xr_d, wp_d, bias_d, id_d, y_d):
    from contextlib import ExitStack

    with ExitStack() as ctx:
        consts = ctx.enter_context(tc.tile_pool(name="consts", bufs=1))
        votes_p = ctx.enter_context(tc.tile_pool(name="votes", bufs=2))
        prod_p = ctx.enter_context(tc.tile_pool(name="prod", bufs=1))
        state_p = ctx.enter_context(tc.tile_pool(name="state", bufs=1))
        out_p = ctx.enter_context(tc.tile_pool(name="outp", bufs=2))
        cpsum = ctx.enter_context(tc.tile_pool(name="cpsum", bufs=1, space="PSUM"))
        tpsum = ctx.enter_context(tc.tile_pool(name="tpsum", bufs=2, space="PSUM"))

        # ---- resident constants ----
        xr_sb = consts.tile([128, NIMG * SLK], F32)
        for img in range(NIMG):
            nc.sync.dma_start(
                xr_sb[:, img * SLK : (img + 1) * SLK], xr_d.ap()[img]
            )
        w_sb = consts.tile([128, NG * COUT], F32)
        for j in range(NG):
            nc.sync.dma_start(w_sb[:, j * COUT : (j + 1) * COUT], wp_d.ap()[j])
        b_sb = consts.tile([128, COUT], F32)
        bias_ap = bias_d.ap()
        bias_bc = bass.AP(
            tensor=bias_ap.tensor, offset=bias_ap.offset, ap=[[0, 128], [1, COUT]]
        )
        nc.sync.dma_start(b_sb[:], bias_bc)
        id_sb = consts.tile([128, 128], F32)
        nc.sync.dma_start(id_sb[:], id_d.ap())

        for blk in range(NBLK):
            off = (2 * blk) * PW

            # ---- conv: votes for 9 images, 128 pixels ----
            cp = [
                cpsum.tile([128, 512], F32, tag=f"cp{i}", name=f"cp{i}_{blk}")
                for i in range(5)
            ]
            for img in range(NIMG):
                out_ap = cp[img // 2][:, (img % 2) * COUT : (img % 2 + 1) * COUT]
                # stationary operand must be single-free-dim: one M=64 matmul
                # group per output row
                for row in range(2):
                    for j, (a, b_) in enumerate(TRANS):
                        d = a * PW + b_
                        base = img * SLK + off + d
                        nc.tensor.matmul(
                            out_ap[row * WD : (row + 1) * WD, :],
                            xr_sb[:, base + row * PW : base + row * PW + WD],
                            w_sb[:, j * COUT : (j + 1) * COUT],
                            start=(j == 0),
                            stop=(j == NG - 1),
                            tile_position=(0, row * WD),
                        )

            votes = votes_p.tile([128, NIMG * COUT], F32)
            for i in range(5):
                w_cols = 512 if i < 4 else COUT
                nc.scalar.copy(votes[:, i * 512 : i * 512 + w_cols], cp[i][:, :w_cols])

            v4 = votes[:, : IC * COUT].rearrange(
                "p (ic nc na) -> p ic nc na", ic=IC, nc=NC_
            )
            v_icr = votes[:, : IC * COUT].rearrange("p (ic r) -> p r ic", ic=IC)
            v_seg = votes[:, : IC * COUT].rearrange("p (s na) -> p s na", na=NA)

            # ---- routing ----
            logits = state_p.tile([128, IC * NC_], F32, tag="logits", bufs=2)
            preact = state_p.tile([128, COUT], F32, tag="preact")
            act = state_p.tile([128, COUT], F32, tag="act", bufs=2)
            nsq = state_p.tile([128, NC_], F32, tag="nsq")
            tsq = state_p.tile([128, NC_], F32, tag="tsq")
            u8 = state_p.tile([128, NC_], F32, tag="u8")
            v8 = state_p.tile([128, NC_], F32, tag="v8")
            f8 = state_p.tile([128, NC_], F32, tag="f8")

            def squash(i_it):
                sq = state_p.tile([128, COUT], F32, tag="sq", name=f"sq_{blk}_{i_it}")
                nc.scalar.square(sq[:], preact[:])
                nc.vector.reduce_sum(
                    out=nsq[:],
                    in_=sq[:].rearrange("p (nc na) -> p nc na", na=NA),
                    axis=AX.X,
                )
                nc.scalar.sqrt(tsq[:], nsq[:])
                nc.vector.tensor_scalar_add(u8[:], nsq[:], 1.0)
                nc.vector.reciprocal(v8[:], u8[:])
                nc.vector.tensor_mul(f8[:], tsq[:], v8[:])
                nc.vector.tensor_mul(
                    act[:].rearrange("p (nc na) -> p nc na", na=NA),
                    preact[:].rearrange("p (nc na) -> p nc na", na=NA),
                    f8[:].unsqueeze(2).broadcast_to((128, NC_, NA)),
                )

            def dist_into(dst_ap):
                prod = prod_p.tile([128, IC * COUT], F32, tag="prod")
                nc.vector.tensor_mul(
                    prod[:].rearrange("p (ic r) -> p ic r", ic=IC),
                    v4.rearrange("p ic nc na -> p ic (nc na)"),
                    act[:].unsqueeze(1).broadcast_to((128, IC, COUT)),
                )
                nc.vector.reduce_sum(
                    out=dst_ap,
                    in_=prod[:].rearrange("p (s na) -> p s na", na=NA),
                    axis=AX.X,
                )

            # iter 1: route == 1/8 exactly -> preact from the conv-of-sums
            nc.vector.scalar_tensor_tensor(
                out=preact[:],
                in0=votes[:, IC * COUT : NIMG * COUT],
                scalar=0.125,
                in1=b_sb[:],
                op0=OP.mult,
                op1=OP.add,
            )
            squash(0)
            dist_into(logits[:].rearrange("p (s na) -> p s na", na=NA).squeeze())

            for it in (1, 2):
                # softmax over nc (free-innermost groups of 8)
                mx = state_p.tile([128, IC], F32, tag="mx")
                ex = state_p.tile([128, IC * NC_], F32, tag="ex")
                sm = state_p.tile([128, IC], F32, tag="sm")
                rc = state_p.tile([128, IC], F32, tag="rc")
                rr = state_p.tile([128, IC * NC_], F32, tag="rr")
                lg3 = logits[:].rearrange("p (ic nc) -> p ic nc", ic=IC)
                nc.vector.reduce_max(out=mx[:], in_=lg3, axis=AX.X)
                nc.vector.tensor_sub(
                    ex[:].rearrange("p (ic nc) -> p ic nc", ic=IC),
                    lg3,
                    mx[:].unsqueeze(2).broadcast_to((128, IC, NC_)),
                )
                nc.scalar.activation(ex[:], ex[:], AF.Exp)
                nc.vector.reduce_sum(
                    out=sm[:],
                    in_=ex[:].rearrange("p (ic nc) -> p ic nc", ic=IC),
                    axis=AX.X,
                )
                nc.vector.reciprocal(rc[:], sm[:])
                nc.vector.tensor_mul(
                    rr[:].rearrange("p (ic nc) -> p ic nc", ic=IC),
                    ex[:].rearrange("p (ic nc) -> p ic nc", ic=IC),
                    rc[:].unsqueeze(2).broadcast_to((128, IC, NC_)),
                )
                # preact = sum_ic r * votes + bias
                prod = prod_p.tile([128, IC * COUT], F32, tag="prod")
                nc.vector.tensor_mul(
                    prod[:].rearrange("p (ic nc na) -> p ic nc na", ic=IC, nc=NC_),
                    v4,
                    rr[:]
                    .rearrange("p (ic nc) -> p ic nc", ic=IC)
                    .unsqueeze(3)
                    .broadcast_to((128, IC, NC_, NA)),
                )
                praw = state_p.tile([128, COUT], F32, tag="sq")
                nc.vector.reduce_sum(
                    out=praw[:],
                    in_=prod[:].rearrange("p (ic r) -> p r ic", ic=IC),
                    axis=AX.X,
                )
                nc.vector.tensor_add(preact[:], praw[:], b_sb[:])
                squash(it)
                if it == 1:
                    dist = state_p.tile([128, IC * NC_], F32, tag="dist")
                    dist_into(dist[:].rearrange("p (s na) -> p s na", na=NA).squeeze())
                    lnew = state_p.tile([128, IC * NC_], F32, tag="logits", bufs=2)
                    nc.vector.tensor_add(lnew[:], logits[:], dist[:])
                    logits = lnew

            # ---- output: transpose [pix, co] -> [co, pix], DMA out ----
            for hf in range(2):
                tp = tpsum.tile([128, 128], F32, tag="tp")
                nc.tensor.transpose(tp[:], act[:, hf * 128 : (hf + 1) * 128], id_sb[:])
                ob = out_p.tile([128, 128], F32, tag="ob")
                nc.scalar.copy(ob[:], tp[:])
                nc.sync.dma_start(
                    y_d.ap()[hf * 128 : (hf + 1) * 128, blk * 128 : (blk + 1) * 128],
                    ob[:],
                )


def kernel(input_tensor, W, b):
    global _PROGRAM
    xr, Wp, bias, ident = _host_prep(input_tensor, W, b)
    if _PROGRAM is None:
        _PROGRAM = _build_program()
    nc = _PROGRAM
    in_maps = [
        {"xr": xr[i], "wp": Wp, "bias": bias, "ident": ident} for i in range(B)
    ]
    res = run_bass_kernel_spmd(nc, in_maps, list(range(B)))
    out = np.stack(
        [res.results[i]["y"].reshape(NC_, NA, H, WD) for i in range(B)], axis=0
    )
    return out.astype(np.float32)


if __name__ == "__main__":
    rng = np.random.default_rng(0)
    x = rng.normal(size=(B, IC, CIN, H, WD)).astype(np.float32)
    W = rng.normal(size=(COUT, CIN, K, K)).astype(np.float32)
    b = np.full((1, 1, NC_, NA), 0.1, np.float32)
    y = kernel(x, W, b)
    print("ok", y.shape, y.dtype)



# revision 12
# speedup vs baseline: 1.8953x; 1.8953x over previous
"""ConvCapsuleLayer Trainium2 kernel.

Per-core (data-parallel over batch B=8 across 8 NeuronCores):
  * 5x5 conv (32->256 ch, 64x64, pad 2) as 8 accumulating K=128 matmuls per
    128-pixel block per image, with the (cin x 4-tap) contraction packed onto
    the 128 PE partitions.  Taps are baked in by staging 4 shifted copies of
    the padded input on partitions (host prep).  A 9th "image" holding
    sum_ic(x) yields sum_ic(votes) by conv linearity, which kills the
    routing-iteration-1 weighted-sum passes (route is exactly 1/8).
  * 3 dynamic-routing iterations fused per 128-pixel block, fp32 throughout
    (bf16 anywhere in the votes path blows up the logits; measured 6.6% L2).
    Layout: partitions = pixels, free = (ic, nc, na); reduces via strided
    innermost-axis tensor_reduce; broadcasts via step-0 APs.
  * Output transposed to [256, pix] via PE transpose, DMA'd to [NC,NA,H,W].
"""

import sys

sys.path.insert(0, "/opt/trn_rl_repo")

import numpy as np

import concourse.bass as bass
import concourse.tile as tile
from concourse import bacc, mybir
from concourse.bass_utils import run_bass_kernel_spmd

F32 = mybir.dt.float32
F32R = mybir.dt.float32r
AX = mybir.AxisListType
OP = mybir.AluOpType
AF = mybir.ActivationFunctionType

B = 8
IC = 8
CIN = 32
NC_ = 8
NA = 32
COUT = NC_ * NA  # 256
H = 64
WD = 64
K = 5
PAD = 2
PW = H + 2 * PAD  # 68
NIMG = IC + 1  # 8 images + sum image
NG = 8  # matmul tap-groups per image
NBLK = 32  # pixel blocks (2 rows of 64 each)
BLKP = 128  # pixels per block

# 4-tap partition shape (dr,dc) and the 8 group translates (a,b).
# Covers all 25 taps of the 5x5 kernel: tap (t,j) = SHAPE[t] + TRANS[j].
SHAPE_T = [(0, 0), (0, 1), (2, 1), (3, 1)]
TRANS = [(0, 0), (0, 2), (1, 1), (1, 3), (1, -1), (0, 3), (2, 0), (2, 2)]

# per-image staged stream length: the full padded image (matmul patch
# slices top out at 62*68 + 138 + 136 = 4490 < 4624).
SLK = PW * PW  # 4624


def _tap_assignment():
    """(t, j) -> (kh, kw) assignment; each of the 25 taps exactly once."""
    assign = {}
    for j, (a, b_) in enumerate(TRANS):
        for t, (dr, dc) in enumerate(SHAPE_T):
            kh, kw = dr + a, dc + b_
            if 0 <= kh < K and 0 <= kw < K and (kh, kw) not in assign:
                assign[(kh, kw)] = (t, j)
    assert len(assign) == K * K, f"tap cover incomplete: {len(assign)}"
    return assign


def _host_prep(input_tensor, W, b):
    x = np.asarray(input_tensor, dtype=np.float32)
    W = np.asarray(W, dtype=np.float32)
    b = np.asarray(b, dtype=np.float32)

    # Padded flat images incl. the sum image, then 4 shifted copies on the
    # partition axis: xr[b, img, t*32+c, i] = xpad[b, img, c, i + s_t].
    xpad = np.zeros((B, NIMG, CIN, PW, PW), np.float32)
    xpad[:, :IC, :, PAD : PAD + H, PAD : PAD + WD] = x
    xpad[:, IC] = xpad[:, :IC].sum(axis=1)
    xflat = np.zeros((B, NIMG, CIN, SLK), np.float32)
    xflat[:, :, :, : PW * PW] = xpad.reshape(B, NIMG, CIN, PW * PW)

    xr = np.zeros((B, NIMG, 128, SLK), np.float32)
    for t, (dr, dc) in enumerate(SHAPE_T):
        s = dr * PW + dc
        xr[:, :, t * CIN : (t + 1) * CIN, : SLK - s] = xflat[:, :, :, s:]

    # Packed weights: Wp[j, t*32+c, co] = W[co, c, kh, kw] for the assigned
    # (t, j) -> (kh, kw); zero elsewhere.
    assign = _tap_assignment()
    Wp = np.zeros((NG, 128, COUT), np.float32)
    for (kh, kw), (t, j) in assign.items():
        Wp[j, t * CIN : (t + 1) * CIN, :] = W[:, :, kh, kw].T

    bias = np.ascontiguousarray(
        np.broadcast_to(b, (1, 1, NC_, NA)).reshape(COUT), dtype=np.float32
    )
    ident = np.eye(128, dtype=np.float32)
    return xr, Wp, bias, ident


_PROGRAM = None


def _build_program():
    nc = bacc.Bacc("TRN2", target_bir_lowering=False, debug=False, num_devices=8)
    xr_d = nc.dram_tensor("xr", [NIMG, 128, SLK], F32R, kind="ExternalInput")
    wp_d = nc.dram_tensor("wp", [NG, 128, COUT], F32R, kind="ExternalInput")
    bias_d = nc.dram_tensor("bias", [COUT], F32, kind="ExternalInput")
    id_d = nc.dram_tensor("ident", [128, 128], F32, kind="ExternalInput")
    y_d = nc.dram_tensor("y", [COUT, H * WD], F32, kind="ExternalOutput")

    with tile.TileContext(nc) as tc:
        _emit(nc, tc, xr_d, wp_d, bias_d, id_d, y_d)
    nc.compile()
    return nc


def _emit(nc, tc, xr_d, wp_d, bias_d, id_d, y_d):
    from contextlib import ExitStack

    with ExitStack() as ctx:
        consts = ctx.enter_context(tc.tile_pool(name="consts", bufs=1))
        votes_p = ctx.enter_context(tc.tile_pool(name="votes", bufs=2))
        prod_p = ctx.enter_context(tc.tile_pool(name="prod", bufs=1))
        state_p = ctx.enter_context(tc.tile_pool(name="state", bufs=1))
        out_p = ctx.enter_context(tc.tile_pool(name="outp", bufs=2))
        cpsum = ctx.enter_context(tc.tile_pool(name="cpsum", bufs=1, space="PSUM"))
        tpsum = ctx.enter_context(tc.tile_pool(name="tpsum", bufs=2, space="PSUM"))

        # ---- resident constants ----
        xr_sb = consts.tile([128, NIMG * SLK], F32R)
        for img in range(NIMG):
            nc.sync.dma_start(
                xr_sb[:, img * SLK : (img + 1) * SLK], xr_d.ap()[img]
            )
        w_sb = consts.tile([128, NG * COUT], F32R)
        for j in range(NG):
            nc.sync.dma_start(w_sb[:, j * COUT : (j + 1) * COUT], wp_d.ap()[j])
        b_sb = consts.tile([128, COUT], F32)
        bias_ap = bias_d.ap()
        bias_bc = bass.AP(
            tensor=bias_ap.tensor, offset=bias_ap.offset, ap=[[0, 128], [1, COUT]]
        )
        nc.sync.dma_start(b_sb[:], bias_bc)
        id_sb = consts.tile([128, 128], F32)
        nc.sync.dma_start(id_sb[:], id_d.ap())

        for blk in range(NBLK):
            off = (2 * blk) * PW

            # ---- conv: votes for 9 images, 128 pixels ----
            # fp32r streams at 1 cyc/row (vs fp32's 4) when the moving dim is
            # >=256, but its PSUM dst must start at partition 0.  So each
            # 64-pixel row gets its own M=64 matmul chain into a partition-0
            # tile; row0 drains to votes[0:64] via scalar copy, row1 to
            # votes[64:128] via a partition-crossing PSUM->SBUF DMA.
            votes = votes_p.tile([128, NIMG * COUT], F32)
            for pair in range(5):
                n_sub = 2 if pair < 4 else 1
                t0 = cpsum.tile(
                    [128, 512], F32, tag=f"r0{pair % 2}", name=f"cp0_{blk}_{pair}"
                )
                t1 = cpsum.tile(
                    [128, 512], F32, tag=f"r1{pair % 2}", name=f"cp1_{blk}_{pair}"
                )
                for sub in range(n_sub):
                    img = pair * 2 + sub
                    for row, tt in ((0, t0), (1, t1)):
                        out_ap = tt[0:64, sub * COUT : (sub + 1) * COUT]
                        for j, (a, b_) in enumerate(TRANS):
                            d = a * PW + b_
                            base = img * SLK + off + d + row * PW
                            nc.tensor.matmul(
                                out_ap,
                                xr_sb[:, base : base + WD],
                                w_sb[:, j * COUT : (j + 1) * COUT],
                                start=(j == 0),
                                stop=(j == NG - 1),
                                tile_position=(0, 0),
                            )
                for sub in range(n_sub):
                    img = pair * 2 + sub
                    c0, c1 = img * COUT, (img + 1) * COUT
                    sl = slice(sub * COUT, (sub + 1) * COUT)
                    stg = out_p.tile(
                        [64, COUT], F32, tag=f"stg{img % 2}", bufs=1,
                        name=f"stg_{blk}_{img}",
                    )
                    nc.scalar.copy(votes[0:64, c0:c1], t0[0:64, sl])
                    nc.scalar.copy(stg[0:64, :], t1[0:64, sl])
                    nc.sync.dma_start(votes[64:128, c0:c1], stg[0:64, :])

            v4 = votes[:, : IC * COUT].rearrange(
                "p (ic nc na) -> p ic nc na", ic=IC, nc=NC_
            )
            v_icr = votes[:, : IC * COUT].rearrange("p (ic r) -> p r ic", ic=IC)
            v_seg = votes[:, : IC * COUT].rearrange("p (s na) -> p s na", na=NA)

            # ---- routing ----
            logits = state_p.tile([128, IC * NC_], F32, tag="logits", bufs=2)
            preact = state_p.tile([128, COUT], F32, tag="preact")
            act = state_p.tile([128, COUT], F32, tag="act", bufs=2)
            nsq = state_p.tile([128, NC_], F32, tag="nsq")
            tsq = state_p.tile([128, NC_], F32, tag="tsq")
            u8 = state_p.tile([128, NC_], F32, tag="u8")
            v8 = state_p.tile([128, NC_], F32, tag="v8")
            f8 = state_p.tile([128, NC_], F32, tag="f8")

            def squash(i_it):
                sq = state_p.tile([128, COUT], F32, tag="sq", name=f"sq_{blk}_{i_it}")
                nc.scalar.square(sq[:], preact[:])
                nc.vector.reduce_sum(
                    out=nsq[:],
                    in_=sq[:].rearrange("p (nc na) -> p nc na", na=NA),
                    axis=AX.X,
                )
                nc.scalar.sqrt(tsq[:], nsq[:])
                nc.vector.tensor_scalar_add(u8[:], nsq[:], 1.0)
                nc.vector.reciprocal(v8[:], u8[:])
                nc.vector.tensor_mul(f8[:], tsq[:], v8[:])
                nc.vector.tensor_mul(
                    act[:].rearrange("p (nc na) -> p nc na", na=NA),
                    preact[:].rearrange("p (nc na) -> p nc na", na=NA),
                    f8[:].unsqueeze(2).broadcast_to((128, NC_, NA)),
                )

            def dist_into(dst_ap):
                prod = prod_p.tile([128, IC * COUT], F32, tag="prod")
                nc.vector.tensor_mul(
                    prod[:].rearrange("p (ic r) -> p ic r", ic=IC),
                    v4.rearrange("p ic nc na -> p ic (nc na)"),
                    act[:].unsqueeze(1).broadcast_to((128, IC, COUT)),
                )
                nc.vector.reduce_sum(
                    out=dst_ap,
                    in_=prod[:].rearrange("p (s na) -> p s na", na=NA),
                    axis=AX.X,
                )

            # iter 1: route == 1/8 exactly -> preact from the conv-of-sums
            nc.vector.scalar_tensor_tensor(
                out=preact[:],
                in0=votes[:, IC * COUT : NIMG * COUT],
                scalar=0.125,
                in1=b_sb[:],
                op0=OP.mult,
                op1=OP.add,
            )
            squash(0)
            dist_into(logits[:].rearrange("p (s na) -> p s na", na=NA).squeeze())

            for it in (1, 2):
                # softmax over nc (free-innermost groups of 8)
                mx = state_p.tile([128, IC], F32, tag="mx")
                ex = state_p.tile([128, IC * NC_], F32, tag="ex")
                sm = state_p.tile([128, IC], F32, tag="sm")
                rc = state_p.tile([128, IC], F32, tag="rc")
                rr = state_p.tile([128, IC * NC_], F32, tag="rr")
                lg3 = logits[:].rearrange("p (ic nc) -> p ic nc", ic=IC)
                nc.vector.reduce_max(out=mx[:], in_=lg3, axis=AX.X)
                nc.vector.tensor_sub(
                    ex[:].rearrange("p (ic nc) -> p ic nc", ic=IC),
                    lg3,
                    mx[:].unsqueeze(2).broadcast_to((128, IC, NC_)),
                )
                nc.scalar.activation(ex[:], ex[:], AF.Exp)
                nc.vector.reduce_sum(
                    out=sm[:],
                    in_=ex[:].rearrange("p (ic nc) -> p ic nc", ic=IC),
                    axis=AX.X,
                )
                nc.vector.reciprocal(rc[:], sm[:])
                nc.vector.tensor_mul(
                    rr[:].rearrange("p (ic nc) -> p ic nc", ic=IC),
                    ex[:].rearrange("p (ic nc) -> p ic nc", ic=IC),
                    rc[:].unsqueeze(2).broadcast_to((128, IC, NC_)),
                )
                # preact = sum_ic r * votes + bias
                prod = prod_p.tile([128, IC * COUT], F32, tag="prod")
                nc.vector.tensor_mul(
                    prod[:].rearrange("p (ic nc na) -> p ic nc na", ic=IC, nc=NC_),
                    v4,
                    rr[:]
                    .rearrange("p (ic nc) -> p ic nc", ic=IC)
                    .unsqueeze(3)
                    .broadcast_to((128, IC, NC_, NA)),
                )
                praw = state_p.tile([128, COUT], F32, tag="sq")
                nc.vector.reduce_sum(
                    out=praw[:],
                    in_=prod[:].rearrange("p (ic r) -> p r ic", ic=IC),
                    axis=AX.X,
                )
                nc.vector.tensor_add(preact[:], praw[:], b_sb[:])
                squash(it)
                if it == 1:
                    dist = state_p.tile([128, IC * NC_], F32, tag="dist")
                    dist_into(dist[:].rearrange("p (s na) -> p s na", na=NA).squeeze())
                    lnew = state_p.tile([128, IC * NC_], F32, tag="logits", bufs=2)
                    nc.vector.tensor_add(lnew[:], logits[:], dist[:])
                    logits = lnew

            # ---- output: transpose [pix, co] -> [co, pix], DMA out ----
            for hf in range(2):
                tp = tpsum.tile([128, 128], F32, tag="tp")
                nc.tensor.transpose(tp[:], act[:, hf * 128 : (hf + 1) * 128], id_sb[:])
                ob = out_p.tile([128, 128], F32, tag="ob")
                nc.scalar.copy(ob[:], tp[:])
                nc.sync.dma_start(
                    y_d.ap()[hf * 128 : (hf + 1) * 128, blk * 128 : (blk + 1) * 128],
                    ob[:],
                )


def kernel(input_tensor, W, b):
    global _PROGRAM
    xr, Wp, bias, ident = _host_prep(input_tensor, W, b)
    if _PROGRAM is None:
        _PROGRAM = _build_program()
    nc = _PROGRAM
    in_maps = [
        {"xr": xr[i], "wp": Wp, "bias": bias, "ident": ident} for i in range(B)
    ]
    res = run_bass_kernel_spmd(nc, in_maps, list(range(B)))
    out = np.stack(
        [res.results[i]["y"].reshape(NC_, NA, H, WD) for i in range(B)], axis=0
    )
    return out.astype(np.float32)


if __name__ == "__main__":
    rng = np.random.default_rng(0)
    x = rng.normal(size=(B, IC, CIN, H, WD)).astype(np.float32)
    W = rng.normal(size=(COUT, CIN, K, K)).astype(np.float32)
    b = np.full((1, 1, NC_, NA), 0.1, np.float32)
    y = kernel(x, W, b)
    print("ok", y.shape, y.dtype)

